# revision 1
# baseline (speedup 1.0000x reference)
"""Trainium2 Bass kernel for a single attention head.

Reference (per batch b):
    q = x @ Wq.T ; k = x @ Wk.T ; v = x @ Wv.T          (x: [S, D])
    scores = (q @ k.T) / sqrt(S)                         ([S, S])
    scores[mask == 0] = -inf  (mask broadcast over query dim)
    out = softmax(scores, -1) @ v

Shapes: B=8, S=2048, D=512, fp32.  Sharding: data-parallel over batch,
one batch element per NeuronCore (8 cores), no collectives.

Per-core dataflow (matmuls in bf16, fp32 PSUM accumulation):
  - host pre-transposes X -> XT [D, S] and W -> WT [D, D] (and casts to
    bf16) so every contraction has its axis on partitions; no on-chip
    transposes anywhere.
  - KT/QT [D, S] and V [S, D] computed on TensorE; QT pre-scaled by
    1/sqrt(S) during its PSUM eviction (DVE).
  - scores computed transposed: ST[k, q] tiles so softmax's key axis is
    the partition axis; ScalarE applies exp(in + bias_k) where
    bias_k = -30000 on masked keys (exp -> 0 exactly), fusing mask,
    scale and softmax numerator into the single PSUM-evicting op.
  - softmax denominator: an N=1 matmul of each E^T chunk against the
    mask column, accumulated alongside the PV matmul (near-free on PE);
    normalization folds into the output's PSUM->SBUF eviction.
  - no max-subtraction needed: scores/sqrt(S) have std ~0.5, |s| < ~3,
    so exp never overflows and softmax is exact without it.
"""

import sys

if "/opt/trn_rl_repo" not in sys.path:
    sys.path.insert(0, "/opt/trn_rl_repo")

import numpy as np

import concourse.bass as bass
import concourse.bacc as bacc
import concourse.mybir as mybir
from concourse.tile import TileContext
from concourse.bass_utils import run_bass_kernel_spmd

B, S, D = 8, 2048, 512
P = 128
NQ = 512                 # q-slab width (matmul moving dim)
DC = D // P              # 4 contraction chunks over d / e
KT = S // P              # 16 key tiles
QS = S // NQ             # 4 q slabs
QT4 = NQ // P            # 4 q tiles per slab
F32 = mybir.dt.float32
BF16 = mybir.dt.bfloat16
SCALE = 1.0 / float(np.sqrt(S))
NEG = -30000.0           # additive mask bias; exp(-30000) == 0.0 in fp32


def _r(ap):
    return ap


def build():
    nc = bacc.Bacc()
    xt = nc.declare_dram_parameter("xt", [D, S], BF16, isOutput=False)
    wqt = nc.declare_dram_parameter("wqt", [D, D], BF16, isOutput=False)
    wkt = nc.declare_dram_parameter("wkt", [D, D], BF16, isOutput=False)
    wvt = nc.declare_dram_parameter("wvt", [D, D], BF16, isOutput=False)
    bias = nc.declare_dram_parameter("bias", [P, KT], F32, isOutput=False)
    mcol = nc.declare_dram_parameter("mcol", [P, KT], BF16, isOutput=False)
    out = nc.declare_dram_parameter("out", [S, D], F32, isOutput=True)

    with TileContext(nc) as tc:
        with (
            tc.tile_pool(name="persist", bufs=1) as persist,
            tc.tile_pool(name="etp", bufs=36) as etp,
            tc.tile_pool(name="outp", bufs=16) as outp,
            tc.tile_pool(name="ps", bufs=4, space="PSUM") as ps_pool,
            tc.tile_pool(name="po", bufs=2, space="PSUM") as po_pool,
            tc.tile_pool(name="pd", bufs=2, space="PSUM") as pd_pool,
        ):
            bias_sb = persist.tile([P, KT], F32, tag="bias", name="bias_sb")
            mcol_sb = persist.tile([P, KT], BF16, tag="mcol", name="mcol_sb")

            qt_sb = [persist.tile([P, S], BF16, tag=f"qt{e}", name=f"qt{e}") for e in range(DC)]
            kt_sb = [persist.tile([P, S], BF16, tag=f"kt{e}", name=f"kt{e}") for e in range(DC)]
            v_sb = [persist.tile([P, D], BF16, tag=f"v{t}", name=f"v{t}") for t in range(KT)]

            with tc.tile_pool(name="ld", bufs=1) as ld:

                def load_w(w, name, defer_to=None):
                    tiles = []
                    for c in range(DC):
                        t = ld.tile([P, D], BF16, tag=f"{name}{c}", name=f"{name}{c}")
                        d = nc.sync.dma_start(out=t, in_=w[c * P : (c + 1) * P, :])
                        if defer_to is not None:
                            defer_to.append(d)
                        tiles.append(t)
                    return tiles

                # K weights first, then X slab-by-slab, so PE can start
                # after ~1 MB of traffic instead of the full input set. DMAs
                # for data not needed until later are held back behind the
                # first K matmul group so the critical first MB gets the full
                # HBM bandwidth (the 16 HWDGE queues otherwise share it
                # fairly).
                from concourse.tile import add_dep_helper

                deferred = []
                wkt_sb = load_w(wkt, "wk")
                xt_sb = [
                    ld.tile([P, S], BF16, tag=f"xt{c}", name=f"xt{c}")
                    for c in range(DC)
                ]
                for s in range(QS):
                    sl = slice(s * NQ, (s + 1) * NQ)
                    for c in range(DC):
                        d = nc.sync.dma_start(
                            out=xt_sb[c][:, sl], in_=xt[c * P : (c + 1) * P, sl]
                        )
                        if s >= 2:
                            deferred.append(d)
                wvt_sb = load_w(wvt, "wv", defer_to=deferred)
                wqt_sb = load_w(wqt, "wq", defer_to=deferred)
                deferred.append(nc.sync.dma_start(out=bias_sb, in_=bias[:, :]))
                deferred.append(nc.sync.dma_start(out=mcol_sb, in_=mcol[:, :]))

                # --- K^T: [e, s] with e on partitions ---
                first_group_last_mm = None
                for e in range(DC):
                    for s in range(QS):
                        sl = slice(s * NQ, (s + 1) * NQ)
                        pk = ps_pool.tile([P, NQ], F32, tag="mm", name="mmps")
                        for c in range(DC):
                            mm = nc.tensor.matmul(
                                pk,
                                _r(wkt_sb[c][:, e * P : (e + 1) * P]),
                                _r(xt_sb[c][:, sl]),
                                start=(c == 0),
                                stop=(c == DC - 1),
                            )
                            if e == 0 and s == 0 and c == DC - 1:
                                first_group_last_mm = mm
                        nc.vector.tensor_copy(out=kt_sb[e][:, sl], in_=pk)

                for d in deferred:
                    add_dep_helper(
                        d.ins, first_group_last_mm.ins,
                        reason="defer non-critical input DMA past first K group",
                    )

                # --- V: [s, e] natural layout ---
                for t in range(KT):
                    pv = ps_pool.tile([P, D], F32, tag="mm", name="mmps")
                    for c in range(DC):
                        nc.tensor.matmul(
                            pv,
                            _r(xt_sb[c][:, t * P : (t + 1) * P]),
                            _r(wvt_sb[c]),
                            start=(c == 0),
                            stop=(c == DC - 1),
                        )
                    nc.vector.tensor_copy(out=v_sb[t], in_=pv)

                # --- Q^T: [e, s], pre-scaled by 1/sqrt(S) ---
                for e in range(DC):
                    for s in range(QS):
                        sl = slice(s * NQ, (s + 1) * NQ)
                        pq = ps_pool.tile([P, NQ], F32, tag="mm", name="mmps")
                        for c in range(DC):
                            nc.tensor.matmul(
                                pq,
                                _r(wqt_sb[c][:, e * P : (e + 1) * P]),
                                _r(xt_sb[c][:, sl]),
                                start=(c == 0),
                                stop=(c == DC - 1),
                            )
                        nc.vector.tensor_scalar_mul(qt_sb[e][:, sl], pq, SCALE)

            # --- attention, one q-slab (512 queries) at a time ---
            for qs in range(QS):
                qsl = slice(qs * NQ, (qs + 1) * NQ)
                ets = []
                for kt_i in range(KT):
                    st = ps_pool.tile([P, NQ], F32, tag="mm", name="mmps")
                    for e in range(DC):
                        nc.tensor.matmul(
                            st,
                            _r(kt_sb[e][:, kt_i * P : (kt_i + 1) * P]),
                            _r(qt_sb[e][:, qsl]),
                            start=(e == 0),
                            stop=(e == DC - 1),
                        )
                    et = etp.tile([P, NQ], BF16, tag="et", name="et")
                    nc.scalar.activation(
                        out=et,
                        in_=st,
                        func=mybir.ActivationFunctionType.Exp,
                        bias=bias_sb[:, kt_i : kt_i + 1],
                        scale=1.0,
                    )
                    ets.append(et)
                for q_i in range(QT4):
                    po = po_pool.tile([P, D], F32, tag="o", name="po")
                    pd = pd_pool.tile([P, 1], F32, tag="d", name="pd")
                    for kt_i in range(KT):
                        lhs = _r(ets[kt_i][:, q_i * P : (q_i + 1) * P])
                        nc.tensor.matmul(
                            po, lhs, _r(v_sb[kt_i]),
                            start=(kt_i == 0), stop=(kt_i == KT - 1),
                        )
                        nc.tensor.matmul(
                            pd, lhs, _r(mcol_sb[:, kt_i : kt_i + 1]),
                            start=(kt_i == 0), stop=(kt_i == KT - 1),
                        )
                    pd_sb = outp.tile([P, 1], F32, tag="pd_sb", name="pd_sb")
                    nc.vector.tensor_copy(out=pd_sb, in_=pd)
                    rec = outp.tile([P, 1], F32, tag="rec", name="rec")
                    nc.vector.reciprocal(out=rec, in_=pd_sb)
                    ot = outp.tile([P, D], F32, tag="ot", name="ot")
                    nc.vector.tensor_scalar_mul(ot, po, rec)
                    q0 = (qs * QT4 + q_i) * P
                    nc.sync.dma_start(out=out[q0 : q0 + P, :], in_=ot)
    return nc


_NC = None


def _get_nc():
    global _NC
    if _NC is None:
        _NC = build()
        if not _NC.is_finalized():
            _NC.finalize()
    return _NC


def make_in_maps(inputs):
    return _make_in_maps(**inputs)


def _make_in_maps(input_vector, mask, Wq, Wk, Wv):
    import ml_dtypes

    bf16 = ml_dtypes.bfloat16
    input_vector = np.asarray(input_vector, dtype=np.float32)
    mask = np.asarray(mask)
    wqt = np.ascontiguousarray(np.asarray(Wq, dtype=np.float32).T).astype(bf16)
    wkt = np.ascontiguousarray(np.asarray(Wk, dtype=np.float32).T).astype(bf16)
    wvt = np.ascontiguousarray(np.asarray(Wv, dtype=np.float32).T).astype(bf16)

    in_maps = []
    for b in range(B):
        xt = np.ascontiguousarray(input_vector[b].T).astype(bf16)  # [D, S]
        m = (mask[b] != 0)
        bias = np.where(m, 0.0, NEG).astype(np.float32).reshape(KT, P).T
        mcol = m.astype(bf16).reshape(KT, P).T
        in_maps.append(
            {
                "xt": xt,
                "wqt": wqt,
                "wkt": wkt,
                "wvt": wvt,
                "bias": np.ascontiguousarray(bias),
                "mcol": np.ascontiguousarray(mcol),
            }
        )
    return in_maps


def kernel(input_vector, mask, Wq, Wk, Wv):
    in_maps = _make_in_maps(input_vector, mask, Wq, Wk, Wv)
    res = run_bass_kernel_spmd(_get_nc(), in_maps, core_ids=list(range(B)))
    return np.stack([res.results[i]["out"] for i in range(B)], axis=0)


if __name__ == "__main__":
    rng = np.random.default_rng(0)
    inputs = {
        "input_vector": rng.standard_normal((B, S, D), dtype=np.float32),
        "mask": rng.integers(0, 2, size=(B, S)).astype(np.int32),
        "Wq": rng.standard_normal((D, D), dtype=np.float32) / np.sqrt(D),
        "Wk": rng.standard_normal((D, D), dtype=np.float32) / np.sqrt(D),
        "Wv": rng.standard_normal((D, D), dtype=np.float32) / np.sqrt(D),
    }
    out = kernel(**inputs)
    print(out.shape, out.dtype)



# revision 2
# speedup vs baseline: 1.5884x; 1.5884x over previous
"""Trainium2 Bass kernel for a single attention head.

Reference (per batch b):
    q = x @ Wq.T ; k = x @ Wk.T ; v = x @ Wv.T          (x: [S, D])
    scores = (q @ k.T) / sqrt(S)                         ([S, S])
    scores[mask == 0] = -inf  (mask broadcast over query dim)
    out = softmax(scores, -1) @ v

Shapes: B=8, S=2048, D=512, fp32.  Sharding: data-parallel over batch,
one batch element per NeuronCore (8 cores), no collectives.

Key optimization (exact, no extra error): masked keys contribute
exp(-inf)=0 to every query, so the host permutes the sequence axis to
put the ~50% active keys first and the kernel only runs K/V projection,
scores, and PV over the first SK (= max active count, padded to 128)
positions.  Queries are processed in the same permuted order and the
host un-permutes the output rows.  bias/mcol kill the <=127 padding
keys (positions count..SK) exactly like masked keys in the dense
version.

Per-core dataflow (matmuls in bf16, fp32 PSUM accumulation):
  - host packs every DRAM input in its exact SBUF layout ([128
    partitions, ...] bf16) so each tensor loads with one or few large
    DMAs; weight/bias DMAs issue on the Scalar queue and x DMAs on the
    Sync queue so the serialized ~600ns-per-DMA issue cost is split
    across two engines at startup.
  - a short burst of junk matmuls runs during the initial DMA wait to
    lift the PE HAM clock gate (1.2 -> 2.4 GHz) before real work.
  - KT/QT [D, *] and V [*, D] computed on TensorE; QT pre-scaled by
    1/sqrt(S) during its PSUM eviction (DVE).
  - scores computed transposed: ST[k, q] tiles so softmax's key axis is
    the partition axis; ScalarE applies exp(in + bias_k) where
    bias_k = -30000 on masked/padding keys (exp -> 0 exactly), fusing
    mask and softmax numerator into the single PSUM-evicting op.
  - softmax denominator: an N=1 matmul of each E^T chunk against the
    mask column, accumulated alongside the PV matmul (~28ns each on
    PE); normalization folds into the output's PSUM->SBUF eviction.
  - no max-subtraction needed: scores/sqrt(S) have std ~0.5, |s| < ~3,
    so exp never overflows and softmax is exact without it.
"""

import sys

if "/opt/trn_rl_repo" not in sys.path:
    sys.path.insert(0, "/opt/trn_rl_repo")

import numpy as np

import concourse.bass as bass
import concourse.bacc as bacc
import concourse.mybir as mybir
from concourse.tile import TileContext
from concourse.bass_utils import run_bass_kernel_spmd

B, S, D = 8, 2048, 512
P = 128
NQ = 512                 # q-slab width (matmul moving dim)
DC = D // P              # 4 contraction chunks over d / e
QS = S // NQ             # 4 q slabs
QT4 = NQ // P            # 4 q tiles per slab
QT = S // P              # 16 output row tiles
F32 = mybir.dt.float32
BF16 = mybir.dt.bfloat16
SCALE = 1.0 / float(np.sqrt(S))
NEG = -30000.0           # additive mask bias; exp(-30000) == 0.0 in fp32
WARMUP_MMS = 18          # junk matmuls to pre-warm the PE HAM clock gate


def _kslabs(sk):
    """Key-axis slab widths for K^T / x-key DMAs (each <=512, >=128)."""
    n, rem = divmod(sk, 384)
    return [384] * n + ([rem] if rem else [])


def build(sk):
    nkt = sk // P            # key tiles
    nc = bacc.Bacc()
    xt = nc.declare_dram_parameter("xt", [P, DC, S], BF16, isOutput=False)
    wqt = nc.declare_dram_parameter("wqt", [P, DC, D], BF16, isOutput=False)
    wkt = nc.declare_dram_parameter("wkt", [P, DC, D], BF16, isOutput=False)
    wvt = nc.declare_dram_parameter("wvt", [P, DC, D], BF16, isOutput=False)
    bias = nc.declare_dram_parameter("bias", [P, nkt], F32, isOutput=False)
    mcol = nc.declare_dram_parameter("mcol", [P, nkt], BF16, isOutput=False)
    out = nc.declare_dram_parameter("out", [QT, P, D], F32, isOutput=True)

    with TileContext(nc) as tc:
        with (
            tc.tile_pool(name="persist", bufs=1) as persist,
            tc.tile_pool(name="etp", bufs=2 * nkt) as etp,
            tc.tile_pool(name="outp", bufs=16) as outp,
            tc.tile_pool(name="ps", bufs=4, space="PSUM") as ps_pool,
            tc.tile_pool(name="po", bufs=2, space="PSUM") as po_pool,
            tc.tile_pool(name="pd", bufs=2, space="PSUM") as pd_pool,
        ):
            bias_sb = persist.tile([P, nkt], F32, tag="bias", name="bias_sb")
            mcol_sb = persist.tile([P, nkt], BF16, tag="mcol", name="mcol_sb")

            xt_sb = persist.tile([P, DC, S], BF16, tag="xt", name="xt_sb")
            qt_sb = persist.tile([P, DC, S], BF16, tag="qt", name="qt_sb")
            kt_sb = persist.tile([P, DC, sk], BF16, tag="kt", name="kt_sb")
            v_sb = persist.tile([P, nkt, D], BF16, tag="v", name="v_sb")
            wq_sb = persist.tile([P, DC, D], BF16, tag="wq", name="wq_sb")
            wk_sb = persist.tile([P, DC, D], BF16, tag="wk", name="wk_sb")
            wv_sb = persist.tile([P, DC, D], BF16, tag="wv", name="wv_sb")
            junk = persist.tile([P, P], BF16, tag="junk", name="junk")

            # --- input DMAs: weights on the Scalar queue, x on Sync ---
            for c in range(DC):
                nc.scalar.dma_start(out=wk_sb[:, c : c + 1, :], in_=wkt[:, c : c + 1, :])
            nc.scalar.dma_start(out=wv_sb, in_=wvt[:, :, :])
            nc.scalar.dma_start(out=wq_sb, in_=wqt[:, :, :])
            nc.scalar.dma_start(out=bias_sb, in_=bias[:, :])
            nc.scalar.dma_start(out=mcol_sb, in_=mcol[:, :])

            kslabs = _kslabs(sk)
            a = 0
            for w in kslabs:
                nc.sync.dma_start(out=xt_sb[:, :, a : a + w], in_=xt[:, :, a : a + w])
                a += w
            while a < S:
                w = min(512, S - a)
                nc.sync.dma_start(out=xt_sb[:, :, a : a + w], in_=xt[:, :, a : a + w])
                a += w

            # --- PE warmup: junk matmuls to lift the HAM clock gate while
            # the first input DMAs are in flight ---
            nc.any.memset(junk, 0)
            for _ in range(WARMUP_MMS):
                pj = po_pool.tile([P, P], F32, tag="o", name="pjunk")
                nc.tensor.matmul(pj, junk, junk, start=True, stop=True)

            # --- K^T: [e, s] with e on partitions, active keys only ---
            a = 0
            for w in kslabs:
                sl = slice(a, a + w)
                a += w
                for e in range(DC):
                    pk = ps_pool.tile([P, w], F32, tag="mm", name="mmps")
                    for c in range(DC):
                        nc.tensor.matmul(
                            pk,
                            wk_sb[:, c : c + 1, e * P : (e + 1) * P],
                            xt_sb[:, c : c + 1, sl],
                            start=(c == 0),
                            stop=(c == DC - 1),
                        )
                    nc.vector.tensor_copy(out=kt_sb[:, e : e + 1, sl], in_=pk)

            # --- V: [s, e] natural layout, active keys only ---
            for t in range(nkt):
                pv = ps_pool.tile([P, D], F32, tag="mm", name="mmps")
                for c in range(DC):
                    nc.tensor.matmul(
                        pv,
                        xt_sb[:, c : c + 1, t * P : (t + 1) * P],
                        wv_sb[:, c : c + 1, :],
                        start=(c == 0),
                        stop=(c == DC - 1),
                    )
                nc.vector.tensor_copy(out=v_sb[:, t : t + 1, :], in_=pv)

            # --- Q^T: [e, s], pre-scaled by 1/sqrt(S), all queries ---
            for s in range(QS):
                sl = slice(s * NQ, (s + 1) * NQ)
                for e in range(DC):
                    pq = ps_pool.tile([P, NQ], F32, tag="mm", name="mmps")
                    for c in range(DC):
                        nc.tensor.matmul(
                            pq,
                            wq_sb[:, c : c + 1, e * P : (e + 1) * P],
                            xt_sb[:, c : c + 1, sl],
                            start=(c == 0),
                            stop=(c == DC - 1),
                        )
                    nc.vector.tensor_scalar_mul(qt_sb[:, e : e + 1, sl], pq, SCALE)

            # --- attention, one q-slab (512 queries) at a time ---
            for qs in range(QS):
                qsl = slice(qs * NQ, (qs + 1) * NQ)
                ets = []
                for kt_i in range(nkt):
                    st = ps_pool.tile([P, NQ], F32, tag="mm", name="mmps")
                    for e in range(DC):
                        nc.tensor.matmul(
                            st,
                            kt_sb[:, e : e + 1, kt_i * P : (kt_i + 1) * P],
                            qt_sb[:, e : e + 1, qsl],
                            start=(e == 0),
                            stop=(e == DC - 1),
                        )
                    et = etp.tile([P, NQ], BF16, tag="et", name="et")
                    nc.scalar.activation(
                        out=et,
                        in_=st,
                        func=mybir.ActivationFunctionType.Exp,
                        bias=bias_sb[:, kt_i : kt_i + 1],
                        scale=1.0,
                    )
                    ets.append(et)
                for q_i in range(QT4):
                    po = po_pool.tile([P, D], F32, tag="o", name="po")
                    pd = pd_pool.tile([P, 1], F32, tag="d", name="pd")
                    for kt_i in range(nkt):
                        lhs = ets[kt_i][:, q_i * P : (q_i + 1) * P]
                        nc.tensor.matmul(
                            po, lhs, v_sb[:, kt_i : kt_i + 1, :],
                            start=(kt_i == 0), stop=(kt_i == nkt - 1),
                        )
                        nc.tensor.matmul(
                            pd, lhs, mcol_sb[:, kt_i : kt_i + 1],
                            start=(kt_i == 0), stop=(kt_i == nkt - 1),
                        )
                    pd_sb = outp.tile([P, 1], F32, tag="pd_sb", name="pd_sb")
                    nc.vector.tensor_copy(out=pd_sb, in_=pd)
                    rec = outp.tile([P, 1], F32, tag="rec", name="rec")
                    nc.vector.reciprocal(out=rec, in_=pd_sb)
                    ot = outp.tile([P, D], F32, tag="ot", name="ot")
                    nc.vector.tensor_scalar_mul(ot, po, rec)
                    t = qs * QT4 + q_i
                    nc.sync.dma_start(out=out[t, :, :], in_=ot)
    return nc


_NC_CACHE = {}


def _get_nc(sk):
    nc = _NC_CACHE.get(sk)
    if nc is None:
        nc = build(sk)
        if not nc.is_finalized():
            nc.finalize()
        _NC_CACHE[sk] = nc
    return nc


def _pack_w(W, bf16):
    # [P, DC, D] with [p, c, j] = W[j, c*128 + p]
    wt = np.ascontiguousarray(np.asarray(W, dtype=np.float32).T)  # [d, j]
    return np.ascontiguousarray(
        wt.reshape(DC, P, D).transpose(1, 0, 2)
    ).astype(bf16)


def make_in_maps(inputs):
    in_maps, _, _ = _prepare(**inputs)
    return in_maps


def _prepare(input_vector, mask, Wq, Wk, Wv):
    import ml_dtypes

    bf16 = ml_dtypes.bfloat16
    x = np.asarray(input_vector, dtype=np.float32)
    mask = np.asarray(mask)

    perms, counts = [], []
    for b in range(B):
        act = np.flatnonzero(mask[b] != 0)
        rest = np.flatnonzero(mask[b] == 0)
        perms.append(np.concatenate([act, rest]))
        counts.append(len(act))
    sk = min(S, max(P, -(-max(counts) // P) * P))
    nkt = sk // P

    wq = _pack_w(Wq, bf16)
    wk = _pack_w(Wk, bf16)
    wv = _pack_w(Wv, bf16)

    in_maps = []
    for b in range(B):
        xtp = x[b].T[:, perms[b]]  # [D, S], columns permuted (active first)
        xtp = np.ascontiguousarray(
            xtp.reshape(DC, P, S).transpose(1, 0, 2)
        ).astype(bf16)
        active = np.arange(sk) < counts[b]
        bias_b = np.where(active, 0.0, NEG).astype(np.float32).reshape(nkt, P).T
        mcol_b = active.astype(bf16).reshape(nkt, P).T
        in_maps.append(
            {
                "xt": xtp,
                "wqt": wq,
                "wkt": wk,
                "wvt": wv,
                "bias": np.ascontiguousarray(bias_b),
                "mcol": np.ascontiguousarray(mcol_b),
            }
        )
    return in_maps, perms, sk


def kernel(input_vector, mask, Wq, Wk, Wv):
    in_maps, perms, sk = _prepare(input_vector, mask, Wq, Wk, Wv)
    res = run_bass_kernel_spmd(_get_nc(sk), in_maps, core_ids=list(range(B)))
    out = np.empty((B, S, D), dtype=np.float32)
    for b in range(B):
        out[b, perms[b], :] = res.results[b]["out"].reshape(S, D)
    return out


if __name__ == "__main__":
    rng = np.random.default_rng(0)
    inputs = {
        "input_vector": rng.standard_normal((B, S, D), dtype=np.float32),
        "mask": rng.integers(0, 2, size=(B, S)).astype(np.int32),
        "Wq": rng.standard_normal((D, D), dtype=np.float32) / np.sqrt(D),
        "Wk": rng.standard_normal((D, D), dtype=np.float32) / np.sqrt(D),
        "Wv": rng.standard_normal((D, D), dtype=np.float32) / np.sqrt(D),
    }
    out = kernel(**inputs)
    print(out.shape, out.dtype)


# revision 6
# speedup vs baseline: 1.6195x; 1.0196x over previous
"""Trainium2 Bass kernel for a single attention head.

Reference (per batch b):
    q = x @ Wq.T ; k = x @ Wk.T ; v = x @ Wv.T          (x: [S, D])
    scores = (q @ k.T) / sqrt(S)                         ([S, S])
    scores[mask == 0] = -inf  (mask broadcast over query dim)
    out = softmax(scores, -1) @ v

Shapes: B=8, S=2048, D=512, fp32.  Sharding: data-parallel over batch,
one batch element per NeuronCore (8 cores), no collectives.

Key optimization (exact, no extra error): masked keys contribute
exp(-inf)=0 to every query, so the host permutes the sequence axis to
put the ~50% active keys first and the kernel only runs K/V projection,
scores, and PV over the first SK (= max active count, padded to 128)
positions.  Queries are processed in the same permuted order and the
host un-permutes the output rows.  bias/mcol kill the <=127 padding
keys (positions count..SK) exactly like masked keys in the dense
version.

Per-core dataflow (matmuls in bf16, fp32 PSUM accumulation):
  - host packs every DRAM input in its exact SBUF layout ([128
    partitions, ...] bf16) so each tensor loads with one or few large
    DMAs; weight/bias DMAs issue on the Scalar queue and x DMAs on the
    Sync queue so the serialized ~600ns-per-DMA issue cost is split
    across two engines at startup.
  - a short burst of junk matmuls runs during the initial DMA wait to
    lift the PE HAM clock gate (1.2 -> 2.4 GHz) before real work.
  - KT/QT [D, *] and V [*, D] computed on TensorE; QT pre-scaled by
    1/sqrt(S) during its PSUM eviction (DVE).
  - scores computed transposed: ST[k, q] tiles so softmax's key axis is
    the partition axis; ScalarE applies exp(in + bias_k) where
    bias_k = -30000 on masked/padding keys (exp -> 0 exactly), fusing
    mask and softmax numerator into the single PSUM-evicting op.
  - softmax denominator: an N=1 matmul of each E^T chunk against the
    mask column, accumulated alongside the PV matmul (~28ns each on
    PE); normalization folds into the output's PSUM->SBUF eviction.
  - no max-subtraction needed: scores/sqrt(S) have std ~0.5, |s| < ~3,
    so exp never overflows and softmax is exact without it.
"""

import sys

if "/opt/trn_rl_repo" not in sys.path:
    sys.path.insert(0, "/opt/trn_rl_repo")

import numpy as np

import concourse.bass as bass
import concourse.bacc as bacc
import concourse.mybir as mybir
from concourse.tile import TileContext
from concourse.bass_utils import run_bass_kernel_spmd

B, S, D = 8, 2048, 512
P = 128
NQ = 512                 # q-slab width (matmul moving dim)
DC = D // P              # 4 contraction chunks over d / e
QS = S // NQ             # 4 q slabs
QT4 = NQ // P            # 4 q tiles per slab
QT = S // P              # 16 output row tiles
F32 = mybir.dt.float32
BF16 = mybir.dt.bfloat16
SCALE = 1.0 / float(np.sqrt(S))
NEG = -30000.0           # additive mask bias; exp(-30000) == 0.0 in fp32
WARMUP_MMS = 24          # junk matmuls bridging the PE from the end of the
                         # engine preamble (~6.7us) to first-data arrival
                         # (~11.5us); keeps the HAM clock gate warm so real
                         # matmuls run at 2.4 GHz from the start


def _kslabs(sk):
    """Key-axis slab widths for K^T / x-key DMAs (each <=512, >=128)."""
    n, rem = divmod(sk, 384)
    return [384] * n + ([rem] if rem else [])


def build(sk):
    nkt = sk // P            # key tiles
    nc = bacc.Bacc()
    xt = nc.declare_dram_parameter("xt", [P, DC, S], BF16, isOutput=False)
    wqt = nc.declare_dram_parameter("wqt", [P, DC, D], BF16, isOutput=False)
    wkt = nc.declare_dram_parameter("wkt", [P, DC, D], BF16, isOutput=False)
    wvt = nc.declare_dram_parameter("wvt", [P, DC, D], BF16, isOutput=False)
    bias = nc.declare_dram_parameter("bias", [P, nkt], F32, isOutput=False)
    mcol = nc.declare_dram_parameter("mcol", [P, nkt], BF16, isOutput=False)
    out = nc.declare_dram_parameter("out", [QT, P, D], F32, isOutput=True)

    with TileContext(nc) as tc:
        with (
            tc.tile_pool(name="persist", bufs=1) as persist,
            tc.tile_pool(name="etp", bufs=2 * nkt) as etp,
            tc.tile_pool(name="outp", bufs=16) as outp,
            tc.tile_pool(name="ps", bufs=4, space="PSUM") as ps_pool,
            tc.tile_pool(name="po", bufs=2, space="PSUM") as po_pool,
            tc.tile_pool(name="pd", bufs=2, space="PSUM") as pd_pool,
        ):
            bias_sb = persist.tile([P, nkt], F32, tag="bias", name="bias_sb")
            mcol_sb = persist.tile([P, nkt], BF16, tag="mcol", name="mcol_sb")

            xt_sb = persist.tile([P, DC, S], BF16, tag="xt", name="xt_sb")
            qt_sb = persist.tile([P, DC, S], BF16, tag="qt", name="qt_sb")
            kt_sb = persist.tile([P, DC, sk], BF16, tag="kt", name="kt_sb")
            v_sb = persist.tile([P, nkt, D], BF16, tag="v", name="v_sb")
            wq_sb = persist.tile([P, DC, D], BF16, tag="wq", name="wq_sb")
            wk_sb = persist.tile([P, DC, D], BF16, tag="wk", name="wk_sb")
            wv_sb = persist.tile([P, DC, D], BF16, tag="wv", name="wv_sb")
            junk = persist.tile([P, P], BF16, tag="junk", name="junk")

            # --- input DMAs: weights on the Scalar queue, x on Sync.
            # The DMA path is cold for the first ~3us (low bandwidth), so
            # only the data the first K^T group needs (wk + x key-slab 0,
            # both split per 128-row chunk for earliest first arrival) is
            # issued eagerly; everything else is held back behind the first
            # K^T matmul group so the critical bytes get the full early
            # bandwidth. ---
            from concourse.tile import add_dep_helper

            deferred = []
            for c in range(DC):
                nc.scalar.dma_start(out=wk_sb[:, c : c + 1, :], in_=wkt[:, c : c + 1, :])
            kslabs = _kslabs(sk)
            a = 0
            for i, w in enumerate(kslabs):
                if i == 0:
                    for c in range(DC):
                        nc.sync.dma_start(
                            out=xt_sb[:, c : c + 1, a : a + w],
                            in_=xt[:, c : c + 1, a : a + w],
                        )
                else:
                    nc.sync.dma_start(
                        out=xt_sb[:, :, a : a + w], in_=xt[:, :, a : a + w]
                    )
                a += w
            deferred.append(nc.scalar.dma_start(out=wv_sb, in_=wvt[:, :, :]))
            deferred.append(nc.scalar.dma_start(out=wq_sb, in_=wqt[:, :, :]))
            deferred.append(nc.scalar.dma_start(out=bias_sb, in_=bias[:, :]))
            deferred.append(nc.scalar.dma_start(out=mcol_sb, in_=mcol[:, :]))
            while a < S:
                w = min(512, S - a)
                deferred.append(
                    nc.sync.dma_start(
                        out=xt_sb[:, :, a : a + w], in_=xt[:, :, a : a + w]
                    )
                )
                a += w

            # --- PE warmup: junk matmuls to lift the HAM clock gate while
            # the first input DMAs are in flight ---
            nc.any.memset(junk, 0)
            for _ in range(WARMUP_MMS):
                pj = po_pool.tile([P, P], F32, tag="o", name="pjunk")
                nc.tensor.matmul(pj, junk, junk, start=True, stop=True)

            # --- K^T: [e, s] with e on partitions, active keys only ---
            a = 0
            first_group_last_mm = None
            for si, w in enumerate(kslabs):
                sl = slice(a, a + w)
                a += w
                for e in range(DC):
                    pk = ps_pool.tile([P, w], F32, tag="mm", name="mmps")
                    for c in range(DC):
                        mm = nc.tensor.matmul(
                            pk,
                            wk_sb[:, c : c + 1, e * P : (e + 1) * P],
                            xt_sb[:, c : c + 1, sl],
                            start=(c == 0),
                            stop=(c == DC - 1),
                        )
                        if si == 0 and e == 0 and c == DC - 1:
                            first_group_last_mm = mm
                    nc.vector.tensor_copy(out=kt_sb[:, e : e + 1, sl], in_=pk)

            for dd in deferred:
                add_dep_helper(
                    dd.ins, first_group_last_mm.ins,
                    reason="defer non-critical input DMA past first K group",
                )

            # --- V: [s, e] natural layout, active keys only ---
            for t in range(nkt):
                pv = ps_pool.tile([P, D], F32, tag="mm", name="mmps")
                for c in range(DC):
                    nc.tensor.matmul(
                        pv,
                        xt_sb[:, c : c + 1, t * P : (t + 1) * P],
                        wv_sb[:, c : c + 1, :],
                        start=(c == 0),
                        stop=(c == DC - 1),
                    )
                nc.vector.tensor_copy(out=v_sb[:, t : t + 1, :], in_=pv)

            # --- Q^T: [e, s], pre-scaled by 1/sqrt(S), all queries ---
            for s in range(QS):
                sl = slice(s * NQ, (s + 1) * NQ)
                for e in range(DC):
                    pq = ps_pool.tile([P, NQ], F32, tag="mm", name="mmps")
                    for c in range(DC):
                        nc.tensor.matmul(
                            pq,
                            wq_sb[:, c : c + 1, e * P : (e + 1) * P],
                            xt_sb[:, c : c + 1, sl],
                            start=(c == 0),
                            stop=(c == DC - 1),
                        )
                    nc.vector.tensor_scalar_mul(qt_sb[:, e : e + 1, sl], pq, SCALE)

            # --- attention, one q-slab (512 queries) at a time ---
            for qs in range(QS):
                qsl = slice(qs * NQ, (qs + 1) * NQ)
                ets = []
                for kt_i in range(nkt):
                    st = ps_pool.tile([P, NQ], F32, tag="mm", name="mmps")
                    for e in range(DC):
                        nc.tensor.matmul(
                            st,
                            kt_sb[:, e : e + 1, kt_i * P : (kt_i + 1) * P],
                            qt_sb[:, e : e + 1, qsl],
                            start=(e == 0),
                            stop=(e == DC - 1),
                        )
                    et = etp.tile([P, NQ], BF16, tag="et", name="et")
                    nc.scalar.activation(
                        out=et,
                        in_=st,
                        func=mybir.ActivationFunctionType.Exp,
                        bias=bias_sb[:, kt_i : kt_i + 1],
                        scale=1.0,
                    )
                    ets.append(et)
                for q_i in range(QT4):
                    po = po_pool.tile([P, D], F32, tag="o", name="po")
                    pd = pd_pool.tile([P, 1], F32, tag="d", name="pd")
                    for kt_i in range(nkt):
                        # pd before po so the denominator's eviction
                        # (copy+reciprocal) overlaps the last po matmul
                        lhs = ets[kt_i][:, q_i * P : (q_i + 1) * P]
                        nc.tensor.matmul(
                            pd, lhs, mcol_sb[:, kt_i : kt_i + 1],
                            start=(kt_i == 0), stop=(kt_i == nkt - 1),
                        )
                        nc.tensor.matmul(
                            po, lhs, v_sb[:, kt_i : kt_i + 1, :],
                            start=(kt_i == 0), stop=(kt_i == nkt - 1),
                        )
                    pd_sb = outp.tile([P, 1], F32, tag="pd_sb", name="pd_sb")
                    nc.vector.tensor_copy(out=pd_sb, in_=pd)
                    rec = outp.tile([P, 1], F32, tag="rec", name="rec")
                    nc.vector.reciprocal(out=rec, in_=pd_sb)
                    ot = outp.tile([P, D], F32, tag="ot", name="ot")
                    nc.vector.tensor_scalar_mul(ot, po, rec)
                    t = qs * QT4 + q_i
                    nc.sync.dma_start(out=out[t, :, :], in_=ot)
    return nc


_NC_CACHE = {}


def _get_nc(sk):
    nc = _NC_CACHE.get(sk)
    if nc is None:
        nc = build(sk)
        if not nc.is_finalized():
            nc.finalize()
        _NC_CACHE[sk] = nc
    return nc


def _pack_w(W, bf16):
    # [P, DC, D] with [p, c, j] = W[j, c*128 + p]
    wt = np.ascontiguousarray(np.asarray(W, dtype=np.float32).T)  # [d, j]
    return np.ascontiguousarray(
        wt.reshape(DC, P, D).transpose(1, 0, 2)
    ).astype(bf16)


def make_in_maps(inputs):
    in_maps, _, _ = _prepare(**inputs)
    return in_maps


def _prepare(input_vector, mask, Wq, Wk, Wv):
    import ml_dtypes

    bf16 = ml_dtypes.bfloat16
    x = np.asarray(input_vector, dtype=np.float32)
    mask = np.asarray(mask)

    perms, counts = [], []
    for b in range(B):
        act = np.flatnonzero(mask[b] != 0)
        rest = np.flatnonzero(mask[b] == 0)
        perms.append(np.concatenate([act, rest]))
        counts.append(len(act))
    sk = min(S, max(P, -(-max(counts) // P) * P))
    nkt = sk // P

    wq = _pack_w(Wq, bf16)
    wk = _pack_w(Wk, bf16)
    wv = _pack_w(Wv, bf16)

    in_maps = []
    for b in range(B):
        xtp = x[b].T[:, perms[b]]  # [D, S], columns permuted (active first)
        xtp = np.ascontiguousarray(
            xtp.reshape(DC, P, S).transpose(1, 0, 2)
        ).astype(bf16)
        active = np.arange(sk) < counts[b]
        bias_b = np.where(active, 0.0, NEG).astype(np.float32).reshape(nkt, P).T
        mcol_b = active.astype(bf16).reshape(nkt, P).T
        in_maps.append(
            {
                "xt": xtp,
                "wqt": wq,
                "wkt": wk,
                "wvt": wv,
                "bias": np.ascontiguousarray(bias_b),
                "mcol": np.ascontiguousarray(mcol_b),
            }
        )
    return in_maps, perms, sk


def kernel(input_vector, mask, Wq, Wk, Wv):
    in_maps, perms, sk = _prepare(input_vector, mask, Wq, Wk, Wv)
    res = run_bass_kernel_spmd(_get_nc(sk), in_maps, core_ids=list(range(B)))
    out = np.empty((B, S, D), dtype=np.float32)
    for b in range(B):
        out[b, perms[b], :] = res.results[b]["out"].reshape(S, D)
    return out


if __name__ == "__main__":
    rng = np.random.default_rng(0)
    inputs = {
        "input_vector": rng.standard_normal((B, S, D), dtype=np.float32),
        "mask": rng.integers(0, 2, size=(B, S)).astype(np.int32),
        "Wq": rng.standard_normal((D, D), dtype=np.float32) / np.sqrt(D),
        "Wk": rng.standard_normal((D, D), dtype=np.float32) / np.sqrt(D),
        "Wv": rng.standard_normal((D, D), dtype=np.float32) / np.sqrt(D),
    }
    out = kernel(**inputs)
    print(out.shape, out.dtype)
